# revision 36
# baseline (speedup 1.0000x reference)
"""Trainium2 Bass kernel for 3-layer CuGraphSAGE on a fanout-8 sampled tree.

The sampled graph produced by fanout-based neighbor sampling is a forest of
B=4096 independent trees (children of parent p are rows [4096+8p, 4096+8p+8)).
We shard by seed block: core c gets 512 seeds plus their full 3-hop subtrees
(4 contiguous row blocks of x, exactly 1/8 of all rows, zero halo).

Per-core pipeline (all activations channel-major [128ch, rows] so the matmul
contraction dim is always the partition dim — no transposes on device).  The
kernel is DMA-roofline bound: 40.4 MB/core must stream from HBM (hop3 fp8 is
33.5 MB of it), so the design keeps the 16 SDMA engines saturated end-to-end:

  * hop3 streams as fp8_e4m3 in 16 x 2 MiB chunks (bufs=3) on the SP HWDGE
    ring, nothing else on that ring; the 8-way mean dilutes fp8's 3.6%
    quantization RMS to ~1e-3 of the final output.  Chunks are de-interleaved
    per 512-parent group on the host (col e*512+p = child e of parent p), so
    the mean-aggregation is accumulating fp8 DoubleRow matmuls (2 MACs/cell/
    cycle) with contiguous rhs slices.
  * hop2 is fp8 and ALSO de-interleaved per 512-hop1-parent group on the
    host, which makes the hop1-parent layer-0 aggregation the same DR-matmul
    shape (it used to be a 4.4us monolithic DVE reduce that serialized the
    pipeline).  hop2 / hop0+hop1 / weights / outputs ride the ACT HWDGE ring.
  * The 1/8 mean is folded into the DR weight scale (fp8 agg weights built
    on-device at DR_SCALE; the self weight is pre-scaled 8*DR_SCALE and the
    PSUM is divided back in the ReLU activation).
  * layer-1 aggregation over hop2's h1 runs as two partial strided DVE
    reduces per block (planes 0-3 after u3, planes 4-7 after u7), so the DVE
    never blocks the PE; layer-1/layer-2 tiles retire one block behind their
    h1 data, and the output is written back in 4 staged DMAs.

Everything for hop1-tile t (h1self, h2, layer-2 self-only output row block)
completes within one block of its hop3 data arriving, so the post-stream tail
is just the seed tiles plus the last hop1 tile.
"""

import os
import numpy as np

# ---------------------------------------------------------------- constants
N_CORES = 8
C = 128                       # channels
B = 4096                      # seeds
S = B // N_CORES              # 512 seeds per core
BLK = [512, 4096, 32768, 262144]          # per-core rows per hop
OFF = [0, 4096, 36864, 299008]            # global start row of each hop block
NPAR1 = BLK[0] + BLK[1]                   # 4608 local layer-1 parents
NH2 = BLK[2]                              # 32768 local hop2 rows
N3 = BLK[3]                               # 262144 local hop3 rows
PT = 512                                  # parents per PSUM tile
NCH = 16                                  # hop3 DMA chunks
CHC = N3 // NCH                           # 16384 cols per chunk (2 MiB)
N_FULL = 2396160
E_FULL = 2392064
OUT_ROWS = 36864
DR_SCALE = 16.0               # fp8 DoubleRow agg-weight scale (see below)

TRACE = os.environ.get("GNN_TRACE", "0") == "1"
V2 = os.environ.get("GNN_V2", "1") == "1"
LAST_RESULT = None

_BASS_CACHE = {}


def _build_bass_v2():
    import concourse.mybir as mybir
    from concourse import bacc
    from concourse.tile import TileContext

    bf16 = mybir.dt.bfloat16
    fp8 = mybir.dt.float8e4
    f32 = mybir.dt.float32
    Relu = mybir.ActivationFunctionType.Relu
    Ident = mybir.ActivationFunctionType.Identity
    AxX = mybir.AxisListType.X
    DRow = mybir.MatmulPerfMode.DoubleRow
    ISCALE = 1.0 / (8.0 * DR_SCALE)

    # Bacc (not raw Bass): its compile() pipeline splits multi-sem sync
    # waits into event semaphores — TRN2 allows at most 1 wait/instruction.
    nc = bacc.Bacc()
    xA = nc.dram_tensor("xA", [C, NPAR1], bf16, kind="ExternalInput")
    xh2d = nc.dram_tensor("xh2", [C, NH2], fp8, kind="ExternalInput")
    x3 = nc.dram_tensor("x3", [C, N3], fp8, kind="ExternalInput")
    # 8 weight blocks + 3 bias columns in one bf16 tensor -> one DMA
    wconsts = nc.dram_tensor("wconsts", [C, 8 * C + 3], bf16,
                             kind="ExternalInput")
    out = nc.dram_tensor("out", [C, NPAR1], bf16, kind="ExternalOutput")
    WIDX = {k: i for i, k in enumerate(
        ("w1a", "w1b", "w2a", "w2b", "w3a", "w3b", "w1bs", "w1a_raw"))}

    with TileContext(nc) as tc:
        with tc.tile_pool(name="const", bufs=1) as constp, \
             tc.tile_pool(name="keep", bufs=1) as keepp, \
             tc.tile_pool(name="dbuf", bufs=5) as dpool, \
             tc.tile_pool(name="hbuf", bufs=2) as hpool, \
             tc.tile_pool(name="xh2buf", bufs=3) as xh2p, \
             tc.tile_pool(name="aggbuf", bufs=4) as aggp, \
             tc.tile_pool(name="ps", bufs=6, space="PSUM") as pp:

            # ---- the SP ring carries the weights (tiny, needed first) and
            # the hop3 stream; everything else rides ACT.  The first two
            # chunks are 1 MiB so block-1 compute starts sooner; all chunk
            # DMAs are emitted up front and the pool buffer semaphores pace
            # them (3-chunk runway).  u-block g's 4096 cols live in chunk
            # CH_OF[g] at offset CH_OFF[g].
            wtile = constp.tile([C, 8 * C + 3], bf16, name="wtile")
            nc.sync.dma_start(wtile[:, :], wconsts[:, :])

            CH_OF, CH_OFF, x3tiles = [], [], []
            for c in range(16):
                t_ = dpool.tile([C, 4096 * 4], fp8, tag="X3", name="x3chunk")
                nc.sync.dma_start(t_[:, :],
                                  x3[:, 16384 * c: 16384 * (c + 1)])
                x3tiles.append(t_)
                for k in range(4):
                    CH_OF.append(t_)
                    CH_OFF.append(k)

            # ACT-ring transfer order tracks first-use times: block-1 u0
            # needs xh2[0:4096] right away; xA0's first 1024 cols feed
            # block-1's hop1-parent tile at u5; the rest of xA0 (seeds'
            # layer-0 children) isn't read until block 2.  hop2 streams
            # through a rolling 3-buffer window (a block's 4096 cols are
            # only read during that block), which keeps the ACT ring from
            # bursting 6 MB up front and frees SBUF for the hop3 runway.
            xh2w = {}

            def xh2_dma(t_):
                w_ = xh2p.tile([C, 4096], fp8, tag="xh2w", name="xh2chunk")
                nc.scalar.dma_start(w_[:, :],
                                    xh2d[:, 4096 * (t_ - 1): 4096 * t_])
                xh2w[t_] = w_

            xh2_dma(1)
            xA0 = keepp.tile([C, NPAR1], bf16, tag="xA0")
            nc.scalar.dma_start(xA0[:, 0:1024], xA[:, 0:1024])
            nc.scalar.dma_start(xA0[:, 1024:NPAR1], xA[:, 1024:NPAR1])

            w = {k: wtile[:, C * i: C * (i + 1)] for k, i in WIDX.items()}
            bt = {f"b{i+1}": wtile[:, 8 * C + i: 8 * C + i + 1]
                  for i in range(3)}
            # fp8 DoubleRow aggregation weight, built on-device: two
            # interleaved copies of DR_SCALE * W1a^T (saves a DMA transfer)
            wdrt = constp.tile([C, 2 * C], fp8, name="wdrt")
            with nc.allow_low_precision(
                    reason="fp8 DoubleRow agg weights; 8-way mean dilutes "
                           "the 3.6% fp8 RMS below tolerance"):
                nc.scalar.activation(wdrt[:, 0:C], w["w1a_raw"], Ident,
                                     scale=DR_SCALE)
                nc.scalar.activation(wdrt[:, C:2 * C], w["w1a_raw"], Ident,
                                     scale=DR_SCALE)
            wdr = wdrt[:, :].rearrange("c (j m) -> c j m", j=2)

            h1self = keepp.tile([C, NPAR1], bf16, tag="h1self")
            l1a0 = keepp.tile([C, S], bf16, tag="l1a0")
            agg0 = keepp.tile([C, S], bf16, tag="agg0")

            def red8(dst_ap, children_ap):
                # dst[c, p] = sum_e children[c, 8p+e]  (natural node order)
                with nc.allow_low_precision(
                        reason="8-way sibling sum is fp32 internal on DVE; "
                               "bf16 rounding of the sum is within tolerance"):
                    nc.vector.reduce_sum(
                        dst_ap,
                        children_ap.rearrange("c (p e) -> c p e", e=8),
                        axis=AxX)

            def drblock(ps, src_ap, self_ap):
                # 8-child fp8 DR mean-agg (+ scaled self) for 512 parents:
                # src is [C, 4096] with plane layout col 512e+p = child e.
                for e in range(4):
                    rhs = src_ap[:, 2 * PT * e: 2 * PT * (e + 1)]
                    nc.tensor.matmul(ps, wdr,
                                     rhs.rearrange("c (j n) -> c j n", j=2),
                                     start=(e == 0), stop=False,
                                     perf_mode=DRow)
                nc.tensor.matmul(ps, w["w1bs"], self_ap,
                                 start=False, stop=True)

            # Pre-gate work: hop1-parent tile 1 and the seeds' layer-0 tile
            # only need xh2w(1)/xA0/wtile, which land ~10us before the first
            # two hop3 chunks.  Running them here fills the PE's gate-wait
            # window and re-warms the HAM clock right before block 1.
            psh1 = pp.tile([C, PT], f32, tag="ps")
            drblock(psh1, xh2w[1][:, :], xA0[:, PT:2 * PT])
            nc.scalar.activation(h1self[:, PT:2 * PT], psh1, Relu,
                                 bias=bt["b1"], scale=ISCALE)
            red8(agg0[:, 0:256], xA0[:, S: S + 2048])
            red8(agg0[:, 256:512], xA0[:, S + 2048: NPAR1])
            ps0 = pp.tile([C, S], f32, tag="ps")
            nc.tensor.matmul(ps0, w["w1a"], agg0[:, :],
                             start=True, stop=False)
            nc.tensor.matmul(ps0, w["w1b"], xA0[:, 0:S],
                             start=False, stop=True)
            nc.scalar.activation(h1self[:, 0:S], ps0, Relu, bias=bt["b1"])
            red8(l1a0[:, 0:64], h1self[:, PT:2 * PT])

            # Start gate: hold the PE until two hop3 chunks are resident.
            # The stream delivers ~0.36 MB/us of hop3 while the PE consumes
            # 0.40 MB/us, so compute must spot the DMA a ~4 MB head start or
            # it starves mid-block; each starve >3.4us also re-throttles the
            # PE clock to 1.2 GHz (HAM), compounding the stall.
            psg = pp.tile([C, PT], f32, tag="ps")
            nc.tensor.matmul(psg[:, 0:1], w["w1a"], x3tiles[1][:, 0:1],
                             start=True, stop=True)

            h2sb = keepp.tile([C, NPAR1], bf16, tag="h2sb")
            ostage = keepp.tile([C, NPAR1], bf16, tag="ostage")
            l2agg = keepp.tile([C, S], bf16, tag="l2agg")

            def ttadd(dst_ap, a_ap, b_ap):
                # pairwise bf16 plane add (contiguous step-1 APs -> DVE 2x
                # mode, ~330ns/512 cols; a strided 8-plane reduce_sum costs
                # 3.6us and lands on the block-boundary critical path)
                with nc.allow_low_precision(
                        reason="pairwise plane adds round to bf16 at tree "
                               "depth 3; ~0.1% on the mean-agg term, within "
                               "tolerance"):
                    nc.vector.tensor_add(dst_ap, a_ap, b_ap)

            def l1mm(t):
                # layer-1 tile t (hop1 parents)
                ps1 = pp.tile([C, PT], f32, tag="ps")
                nc.tensor.matmul(ps1, w["w2a"], aggs[t][:, :],
                                 start=True, stop=False)
                nc.tensor.matmul(ps1, w["w2b"],
                                 h1self[:, PT * t: PT * (t + 1)],
                                 start=False, stop=True)
                nc.scalar.activation(h2sb[:, PT * t: PT * (t + 1)], ps1,
                                     Relu, bias=bt["b2"])

            def l2self(t):
                pso = pp.tile([C, PT], f32, tag="ps")
                nc.tensor.matmul(pso, w["w3b"],
                                 h2sb[:, PT * t: PT * (t + 1)],
                                 start=True, stop=True)
                nc.scalar.activation(ostage[:, PT * t: PT * (t + 1)], pso,
                                     Relu, bias=bt["b3"])

            aggs = {}
            for t in range(1, 9):
                h1 = hpool.tile([C, 8 * PT], bf16, tag="h1tmp")
                tp = [aggp.tile([C, PT], bf16, tag="tpair", name=f"tp{i}")
                      for i in range(6)]
                agg = aggp.tile([C, PT], bf16, tag="agg")
                aggs[t] = agg
                xh2c = xh2w.pop(t)
                for u in range(8):
                    g = 8 * (t - 1) + u
                    xc, off = CH_OF[g], CH_OFF[g]
                    cbase = 8 * PT * off
                    psu = pp.tile([C, PT], f32, tag="ps")
                    drblock(psu, xc[:, cbase: cbase + 8 * PT],
                            xh2c[:, PT * u: PT * (u + 1)])
                    nc.scalar.activation(h1[:, PT * u: PT * (u + 1)], psu,
                                         Relu, bias=bt["b1"], scale=ISCALE)

                    # --- interleaved small work (deps satisfied earlier) ---
                    if u == 0 and t <= 6:
                        xh2_dma(t + 2)       # hop2 window, 2 blocks ahead
                        if t == 1:
                            xh2_dma(2)
                    elif u == 1:
                        ttadd(tp[0][:, :], h1[:, 0:PT], h1[:, PT:2 * PT])
                    elif u == 2:
                        if t >= 2:
                            l1mm(t - 1)
                    elif u == 3:
                        ttadd(tp[1][:, :], tp[0][:, :], h1[:, 2 * PT:3 * PT])
                        ttadd(tp[2][:, :], tp[1][:, :], h1[:, 3 * PT:4 * PT])
                    elif u == 4 and t >= 2:
                        l2self(t - 1)
                        if t == 5:
                            nc.sync.dma_start(out[:, PT:5 * PT],
                                              ostage[:, PT:5 * PT])
                        elif t == 8:
                            nc.sync.dma_start(out[:, 5 * PT:8 * PT],
                                              ostage[:, 5 * PT:8 * PT])
                    elif u == 5:
                        ttadd(tp[3][:, :], tp[2][:, :], h1[:, 4 * PT:5 * PT])
                        ttadd(tp[4][:, :], tp[3][:, :], h1[:, 5 * PT:6 * PT])
                        if t >= 2:
                            # hop1-parent layer-0 tile t: children are hop2
                            # chunk t (pi plane order), self is xA0
                            psh = pp.tile([C, PT], f32, tag="ps")
                            drblock(psh, xh2c[:, :],
                                    xA0[:, PT * t: PT * (t + 1)])
                            nc.scalar.activation(
                                h1self[:, PT * t: PT * (t + 1)], psh, Relu,
                                bias=bt["b1"], scale=ISCALE)
                    elif u == 6:
                        if t >= 2:
                            # layer-2 agg piece for seeds with children in
                            # tile t-1 (h2sb tile t-1 was written at u2)
                            red8(l2agg[:, 64 * (t - 2): 64 * (t - 1)],
                                 h2sb[:, PT * (t - 1): PT * t])
                            # seeds' layer-1 agg piece over hop1 tile t
                            red8(l1a0[:, 64 * (t - 1): 64 * t],
                                 h1self[:, PT * t: PT * (t + 1)])
                    elif u == 7:
                        ttadd(tp[5][:, :], tp[4][:, :], h1[:, 6 * PT:7 * PT])
                        ttadd(agg[:, :], tp[5][:, :], h1[:, 7 * PT:8 * PT])
                        if t == 8:
                            # seeds' layer-1 (l1a0 piece 8 reduced at u6)
                            psA = pp.tile([C, S], f32, tag="ps")
                            nc.tensor.matmul(psA, w["w2a"], l1a0[:, :],
                                             start=True, stop=False)
                            nc.tensor.matmul(psA, w["w2b"], h1self[:, 0:S],
                                             start=False, stop=True)
                            nc.scalar.activation(h2sb[:, 0:S], psA, Relu,
                                                 bias=bt["b2"])

            # ---------------- tail ----------------
            l1mm(8)
            l2self(8)
            red8(l2agg[:, 448:512], h2sb[:, 8 * PT: 9 * PT])
            # seeds' layer 2: full agg + self
            ps2 = pp.tile([C, S], f32, tag="ps")
            nc.tensor.matmul(ps2, w["w3a"], l2agg[:, :],
                             start=True, stop=False)
            nc.tensor.matmul(ps2, w["w3b"], h2sb[:, 0:S],
                             start=False, stop=True)
            nc.scalar.activation(ostage[:, 0:S], ps2, Relu, bias=bt["b3"])
            nc.sync.dma_start(out[:, 8 * PT:NPAR1], ostage[:, 8 * PT:NPAR1])
            nc.sync.dma_start(out[:, 0:S], ostage[:, 0:S])

    nc.compile()
    return nc


def _get_bass(key):
    if key not in _BASS_CACHE:
        _BASS_CACHE[key] = _build_bass_v2()
    return _BASS_CACHE[key]


def _edge_is_tree(edge):
    if edge.shape != (2, E_FULL):
        return False
    ar = np.arange(E_FULL, dtype=np.int64)
    return (np.array_equal(edge[0], (B + ar).astype(np.int32))
            and np.array_equal(edge[1], (ar // 8).astype(np.int32)))


def _fallback(x, edge, W1, b1, W2, b2, W3, b3):
    # General (structure-agnostic) CPU implementation; only used if the
    # inputs are not the fanout-8 tree this kernel is specialized for.
    sizes = [(N_FULL, E_FULL), (299008, 294912), (36864, 32768)]
    params = [(W1, b1), (W2, b2), (W3, b3)]
    x = x.astype(np.float32)
    for (n, e), (Wl, bl) in zip(sizes, params):
        src = edge[0, :e].astype(np.int64)
        dst = edge[1, :e].astype(np.int64)
        x = x[:n]
        agg = np.zeros((n, x.shape[1]), np.float32)
        np.add.at(agg, dst, x[src])
        deg = np.bincount(dst, minlength=n).astype(np.float32)
        agg /= np.maximum(deg, 1.0)[:, None]
        x = np.maximum(np.concatenate([agg, x], axis=1) @ Wl.T + bl, 0.0)
    return x


# pi permutation: hop2 local row 4096g + 8p + e  ->  4096g + 512e + p
# (de-interleave per 512-hop1-parent group, so the hop1-parent layer-0
# aggregation is 4 accumulating DR matmuls over contiguous plane pairs).
# Both reorders below are expressed as pure reshapes/transposes of that
# permutation (verified equivalent to the explicit index-gather form).


def kernel(**inputs):
    global LAST_RESULT
    import ml_dtypes

    x = np.asarray(inputs["x"])
    edge = np.asarray(inputs["edge"])
    W = [np.asarray(inputs[k], dtype=np.float32) for k in ("W1", "W2", "W3")]
    bias = [np.asarray(inputs[k], dtype=np.float32) for k in ("b1", "b2", "b3")]

    if x.shape != (N_FULL, C) or not _edge_is_tree(edge):
        return _fallback(x, edge, W[0], bias[0], W[1], bias[1], W[2], bias[2])

    from concourse.bass_utils import run_bass_kernel_spmd

    bf = ml_dtypes.bfloat16
    f8 = ml_dtypes.float8_e4m3fn          # bit-compatible with TRN e4m3 < 240
    x = np.ascontiguousarray(x, dtype=np.float32)

    wblocks = []
    for li in range(3):
        wblocks.append((W[li][:, :C] / 8.0).T)     # agg part, mean folded in
        wblocks.append(W[li][:, C:].T)             # self part
    # DoubleRow path: the fp8 agg weight (built on-device from w1a_raw,
    # scaled by DR_SCALE to sit in e4m3's normal range) pairs with a self
    # weight scaled by 8*DR_SCALE; the PSUM is divided back by 8*DR_SCALE
    # in the activation (ReLU is positively homogeneous), which also
    # restores the /8 of the mean.
    wblocks.append(W[0][:, C:].T * (8.0 * DR_SCALE))          # w1bs
    wblocks.append(W[0][:, :C].T)                             # w1a_raw
    wblocks.append(np.stack(bias, axis=1))                    # 3 bias cols
    wconsts = np.ascontiguousarray(np.concatenate(wblocks, axis=1)).astype(bf)

    in_maps = []
    for c in range(N_CORES):
        xloc = [x[OFF[h] + BLK[h] * c: OFF[h] + BLK[h] * (c + 1)]
                for h in range(4)]
        xAc = np.ascontiguousarray(np.concatenate(xloc[:2], axis=0).T).astype(bf)
        # hop2 in pi order
        xh2pi = xloc[2].reshape(8, PT, 8, C).transpose(0, 2, 1, 3)
        xh2c = np.ascontiguousarray(xh2pi.reshape(-1, C).T).astype(f8)
        # hop3 grouped by pi-ordered hop2 parent, then de-interleaved per
        # 512-parent group: col 4096k + 512e + p  <-  child e of pi-parent
        # 512k + p
        x3n = xloc[3].reshape(8, PT, 8, 8, C).transpose(0, 2, 3, 1, 4)
        x3c = np.ascontiguousarray(x3n.reshape(-1, C).T).astype(f8)
        in_maps.append({"xA": xAc, "xh2": xh2c, "x3": x3c,
                        "wconsts": wconsts})

    nc = _get_bass("v2")
    res = run_bass_kernel_spmd(nc, in_maps, list(range(N_CORES)), trace=TRACE)
    LAST_RESULT = res

    out = np.empty((OUT_ROWS, C), np.float32)
    for c in range(N_CORES):
        oc = np.asarray(res.results[c]["out"]).astype(np.float32)
        out[S * c: S * (c + 1)] = oc[:, :S].T
        out[B + 8 * S * c: B + 8 * S * (c + 1)] = oc[:, S:].T
    return out


# revision 37
# speedup vs baseline: 1.1265x; 1.1265x over previous
"""Trainium2 Bass kernel for 3-layer CuGraphSAGE on a fanout-8 sampled tree.

The sampled graph produced by fanout-based neighbor sampling is a forest of
B=4096 independent trees (children of parent p are rows [4096+8p, 4096+8p+8)).
We shard by seed block: core c gets 512 seeds plus their full 3-hop subtrees
(4 contiguous row blocks of x, exactly 1/8 of all rows, zero halo).

Per-core pipeline (all activations channel-major [128ch, rows] so the matmul
contraction dim is always the partition dim — no transposes on device).  The
kernel is DMA-roofline bound: 40.4 MB/core must stream from HBM (hop3 fp8 is
33.5 MB of it), so the design keeps the 16 SDMA engines saturated end-to-end:

  * hop3 streams as fp8_e4m3 in 16 x 2 MiB chunks (bufs=3) on the SP HWDGE
    ring, nothing else on that ring; the 8-way mean dilutes fp8's 3.6%
    quantization RMS to ~1e-3 of the final output.  Chunks are de-interleaved
    per 512-parent group on the host (col e*512+p = child e of parent p), so
    the mean-aggregation is accumulating fp8 DoubleRow matmuls (2 MACs/cell/
    cycle) with contiguous rhs slices.
  * hop2 is fp8 and ALSO de-interleaved per 512-hop1-parent group on the
    host, which makes the hop1-parent layer-0 aggregation the same DR-matmul
    shape (it used to be a 4.4us monolithic DVE reduce that serialized the
    pipeline).  hop2 / hop0+hop1 / weights / outputs ride the ACT HWDGE ring.
  * The 1/8 mean is folded into the DR weight scale (fp8 agg weights built
    on-device at DR_SCALE; the self weight is pre-scaled 8*DR_SCALE and the
    PSUM is divided back in the ReLU activation).
  * layer-1 aggregation over hop2's h1 runs as two partial strided DVE
    reduces per block (planes 0-3 after u3, planes 4-7 after u7), so the DVE
    never blocks the PE; layer-1/layer-2 tiles retire one block behind their
    h1 data, and the output is written back in 4 staged DMAs.

Everything for hop1-tile t (h1self, h2, layer-2 self-only output row block)
completes within one block of its hop3 data arriving, so the post-stream tail
is just the seed tiles plus the last hop1 tile.
"""

import os
import numpy as np

# ---------------------------------------------------------------- constants
N_CORES = 8
C = 128                       # channels
B = 4096                      # seeds
S = B // N_CORES              # 512 seeds per core
BLK = [512, 4096, 32768, 262144]          # per-core rows per hop
OFF = [0, 4096, 36864, 299008]            # global start row of each hop block
NPAR1 = BLK[0] + BLK[1]                   # 4608 local layer-1 parents
NH2 = BLK[2]                              # 32768 local hop2 rows
N3 = BLK[3]                               # 262144 local hop3 rows
PT = 512                                  # parents per PSUM tile
NCH = 16                                  # hop3 DMA chunks
CHC = N3 // NCH                           # 16384 cols per chunk (2 MiB)
N_FULL = 2396160
E_FULL = 2392064
OUT_ROWS = 36864
DR_SCALE = 16.0               # fp8 DoubleRow agg-weight scale (see below)

TRACE = os.environ.get("GNN_TRACE", "0") == "1"
V2 = os.environ.get("GNN_V2", "1") == "1"
LAST_RESULT = None

_BASS_CACHE = {}


def _build_bass_v2():
    import concourse.mybir as mybir
    from concourse import bacc
    from concourse.tile import TileContext

    bf16 = mybir.dt.bfloat16
    fp8 = mybir.dt.float8e4
    f32 = mybir.dt.float32
    Relu = mybir.ActivationFunctionType.Relu
    Ident = mybir.ActivationFunctionType.Identity
    AxX = mybir.AxisListType.X
    DRow = mybir.MatmulPerfMode.DoubleRow
    ISCALE = 1.0 / (8.0 * DR_SCALE)

    # Bacc (not raw Bass): its compile() pipeline splits multi-sem sync
    # waits into event semaphores — TRN2 allows at most 1 wait/instruction.
    nc = bacc.Bacc()
    xA = nc.dram_tensor("xA", [C, NPAR1], bf16, kind="ExternalInput")
    xh2d = nc.dram_tensor("xh2", [C, NH2], fp8, kind="ExternalInput")
    x3 = nc.dram_tensor("x3", [C, N3], fp8, kind="ExternalInput")
    # 8 weight blocks + 3 bias columns in one bf16 tensor -> one DMA
    wconsts = nc.dram_tensor("wconsts", [C, 8 * C + 3], bf16,
                             kind="ExternalInput")
    out = nc.dram_tensor("out", [C, NPAR1], bf16, kind="ExternalOutput")
    WIDX = {k: i for i, k in enumerate(
        ("w1a", "w1b", "w2a", "w2b", "w3a", "w3b", "w1bs", "w1a_raw"))}

    with TileContext(nc) as tc:
        with tc.tile_pool(name="const", bufs=1) as constp, \
             tc.tile_pool(name="keep", bufs=1) as keepp, \
             tc.tile_pool(name="dbuf", bufs=5) as dpool, \
             tc.tile_pool(name="hbuf", bufs=2) as hpool, \
             tc.tile_pool(name="xh2buf", bufs=3) as xh2p, \
             tc.tile_pool(name="aggbuf", bufs=4) as aggp, \
             tc.tile_pool(name="ps", bufs=6, space="PSUM") as pp:

            # ---- the SP ring carries the weights (tiny, needed first) and
            # the hop3 stream; everything else rides ACT.  The first two
            # chunks are 1 MiB so block-1 compute starts sooner; all chunk
            # DMAs are emitted up front and the pool buffer semaphores pace
            # them (3-chunk runway).  u-block g's 4096 cols live in chunk
            # CH_OF[g] at offset CH_OFF[g].
            wtile = constp.tile([C, 8 * C + 3], bf16, name="wtile")
            nc.sync.dma_start(wtile[:, :], wconsts[:, :])

            CH_OF, CH_OFF, x3tiles = [], [], []
            for c in range(16):
                t_ = dpool.tile([C, 4096 * 4], fp8, tag="X3", name="x3chunk")
                nc.sync.dma_start(t_[:, :],
                                  x3[:, 16384 * c: 16384 * (c + 1)])
                x3tiles.append(t_)
                for k in range(4):
                    CH_OF.append(t_)
                    CH_OFF.append(k)

            # ACT-ring transfer order tracks first-use times: block-1 u0
            # needs xh2[0:4096] right away; xA0's first 1024 cols feed
            # block-1's hop1-parent tile at u5; the rest of xA0 (seeds'
            # layer-0 children) isn't read until block 2.  hop2 streams
            # through a rolling 3-buffer window (a block's 4096 cols are
            # only read during that block), which keeps the ACT ring from
            # bursting 6 MB up front and frees SBUF for the hop3 runway.
            xh2w = {}

            def xh2_dma(t_):
                w_ = xh2p.tile([C, 4096], fp8, tag="xh2w", name="xh2chunk")
                nc.scalar.dma_start(w_[:, :],
                                    xh2d[:, 4096 * (t_ - 1): 4096 * t_])
                xh2w[t_] = w_

            xh2_dma(1)
            xA0 = keepp.tile([C, NPAR1], bf16, tag="xA0")
            nc.scalar.dma_start(xA0[:, 0:1024], xA[:, 0:1024])
            nc.scalar.dma_start(xA0[:, 1024:NPAR1], xA[:, 1024:NPAR1])

            w = {k: wtile[:, C * i: C * (i + 1)] for k, i in WIDX.items()}
            bt = {f"b{i+1}": wtile[:, 8 * C + i: 8 * C + i + 1]
                  for i in range(3)}
            # fp8 DoubleRow aggregation weight, built on-device: two
            # interleaved copies of DR_SCALE * W1a^T (saves a DMA transfer)
            wdrt = constp.tile([C, 2 * C], fp8, name="wdrt")
            with nc.allow_low_precision(
                    reason="fp8 DoubleRow agg weights; 8-way mean dilutes "
                           "the 3.6% fp8 RMS below tolerance"):
                nc.scalar.activation(wdrt[:, 0:C], w["w1a_raw"], Ident,
                                     scale=DR_SCALE)
                nc.scalar.activation(wdrt[:, C:2 * C], w["w1a_raw"], Ident,
                                     scale=DR_SCALE)
            wdr = wdrt[:, :].rearrange("c (j m) -> c j m", j=2)

            # Start gate: hold the PE until two hop3 chunks are resident.
            # The stream delivers ~0.36 MB/us of hop3 while the PE consumes
            # 0.40 MB/us, so compute must spot the DMA a ~4 MB head start or
            # it starves mid-block; each starve >3.4us also re-throttles the
            # PE clock to 1.2 GHz (HAM), compounding the stall.
            psg = pp.tile([C, PT], f32, tag="ps")
            nc.tensor.matmul(psg[:, 0:1], w["w1a"], x3tiles[1][:, 0:1],
                             start=True, stop=True)

            h1self = keepp.tile([C, NPAR1], bf16, tag="h1self")
            h2sb = keepp.tile([C, NPAR1], bf16, tag="h2sb")
            ostage = keepp.tile([C, NPAR1], bf16, tag="ostage")
            l1a0 = keepp.tile([C, S], bf16, tag="l1a0")
            l2agg = keepp.tile([C, S], bf16, tag="l2agg")
            agg0 = keepp.tile([C, S], bf16, tag="agg0")

            def red8(dst_ap, children_ap):
                # dst[c, p] = sum_e children[c, 8p+e]  (natural node order)
                with nc.allow_low_precision(
                        reason="8-way sibling sum is fp32 internal on DVE; "
                               "bf16 rounding of the sum is within tolerance"):
                    nc.vector.reduce_sum(
                        dst_ap,
                        children_ap.rearrange("c (p e) -> c p e", e=8),
                        axis=AxX)

            def ttadd(dst_ap, a_ap, b_ap):
                # pairwise bf16 plane add (contiguous step-1 APs -> DVE 2x
                # mode, ~330ns/512 cols; a strided 8-plane reduce_sum costs
                # 3.6us and lands on the block-boundary critical path)
                with nc.allow_low_precision(
                        reason="pairwise plane adds round to bf16 at tree "
                               "depth 3; ~0.1% on the mean-agg term, within "
                               "tolerance"):
                    nc.vector.tensor_add(dst_ap, a_ap, b_ap)

            def drblock(ps, src_ap, self_ap):
                # 8-child fp8 DR mean-agg (+ scaled self) for 512 parents:
                # src is [C, 4096] with plane layout col 512e+p = child e.
                for e in range(4):
                    rhs = src_ap[:, 2 * PT * e: 2 * PT * (e + 1)]
                    nc.tensor.matmul(ps, wdr,
                                     rhs.rearrange("c (j n) -> c j n", j=2),
                                     start=(e == 0), stop=False,
                                     perf_mode=DRow)
                nc.tensor.matmul(ps, w["w1bs"], self_ap,
                                 start=False, stop=True)

            def l1mm(t):
                # layer-1 tile t (hop1 parents)
                ps1 = pp.tile([C, PT], f32, tag="ps")
                nc.tensor.matmul(ps1, w["w2a"], aggs[t][:, :],
                                 start=True, stop=False)
                nc.tensor.matmul(ps1, w["w2b"],
                                 h1self[:, PT * t: PT * (t + 1)],
                                 start=False, stop=True)
                nc.scalar.activation(h2sb[:, PT * t: PT * (t + 1)], ps1,
                                     Relu, bias=bt["b2"])

            def l2self(t):
                pso = pp.tile([C, PT], f32, tag="ps")
                nc.tensor.matmul(pso, w["w3b"],
                                 h2sb[:, PT * t: PT * (t + 1)],
                                 start=True, stop=True)
                nc.scalar.activation(ostage[:, PT * t: PT * (t + 1)], pso,
                                     Relu, bias=bt["b3"])

            aggs = {}
            for t in range(1, 9):
                h1 = hpool.tile([C, 8 * PT], bf16, tag="h1tmp")
                tp = [aggp.tile([C, PT], bf16, tag="tpair", name=f"tp{i}")
                      for i in range(6)]
                agg = aggp.tile([C, PT], bf16, tag="agg")
                aggs[t] = agg
                xh2c = xh2w.pop(t)
                for u in range(8):
                    g = 8 * (t - 1) + u
                    xc, off = CH_OF[g], CH_OFF[g]
                    cbase = 8 * PT * off
                    psu = pp.tile([C, PT], f32, tag="ps")
                    drblock(psu, xc[:, cbase: cbase + 8 * PT],
                            xh2c[:, PT * u: PT * (u + 1)])
                    nc.scalar.activation(h1[:, PT * u: PT * (u + 1)], psu,
                                         Relu, bias=bt["b1"], scale=ISCALE)

                    # --- interleaved small work (deps satisfied earlier) ---
                    if u == 0 and t <= 6:
                        xh2_dma(t + 2)       # hop2 window, 2 blocks ahead
                        if t == 1:
                            xh2_dma(2)
                    elif u == 1:
                        ttadd(tp[0][:, :], h1[:, 0:PT], h1[:, PT:2 * PT])
                        if t == 2:
                            # seeds' layer-0 agg (children = hop1 rows of
                            # xA0, which fully arrive during block 1)
                            red8(agg0[:, 0:256], xA0[:, S: S + 2048])
                    elif u == 2:
                        if t >= 2:
                            l1mm(t - 1)
                        if t == 2:
                            red8(agg0[:, 256:512], xA0[:, S + 2048: NPAR1])
                    elif u == 3:
                        ttadd(tp[1][:, :], tp[0][:, :], h1[:, 2 * PT:3 * PT])
                        ttadd(tp[2][:, :], tp[1][:, :], h1[:, 3 * PT:4 * PT])
                    elif u == 4 and t >= 2:
                        l2self(t - 1)
                        if t == 5:
                            nc.sync.dma_start(out[:, PT:5 * PT],
                                              ostage[:, PT:5 * PT])
                        elif t == 8:
                            nc.sync.dma_start(out[:, 5 * PT:8 * PT],
                                              ostage[:, 5 * PT:8 * PT])
                    elif u == 5:
                        ttadd(tp[3][:, :], tp[2][:, :], h1[:, 4 * PT:5 * PT])
                        ttadd(tp[4][:, :], tp[3][:, :], h1[:, 5 * PT:6 * PT])
                        # hop1-parent layer-0 tile t: children are hop2
                        # chunk t (pi plane order), self is xA0
                        psh = pp.tile([C, PT], f32, tag="ps")
                        drblock(psh, xh2c[:, :],
                                xA0[:, PT * t: PT * (t + 1)])
                        nc.scalar.activation(
                            h1self[:, PT * t: PT * (t + 1)], psh, Relu,
                            bias=bt["b1"], scale=ISCALE)
                    elif u == 6:
                        if t >= 2:
                            # layer-2 agg piece for seeds with children in
                            # tile t-1 (h2sb tile t-1 was written at u2)
                            red8(l2agg[:, 64 * (t - 2): 64 * (t - 1)],
                                 h2sb[:, PT * (t - 1): PT * t])
                        # seeds' layer-1 agg piece over hop1 tile t's h1
                        red8(l1a0[:, 64 * (t - 1): 64 * t],
                             h1self[:, PT * t: PT * (t + 1)])
                        if t == 2:
                            # seeds' layer-0 tile (agg0 reduced at u1/u2)
                            ps0 = pp.tile([C, S], f32, tag="ps")
                            nc.tensor.matmul(ps0, w["w1a"], agg0[:, :],
                                             start=True, stop=False)
                            nc.tensor.matmul(ps0, w["w1b"], xA0[:, 0:S],
                                             start=False, stop=True)
                            nc.scalar.activation(h1self[:, 0:S], ps0, Relu,
                                                 bias=bt["b1"])
                    elif u == 7:
                        ttadd(tp[5][:, :], tp[4][:, :], h1[:, 6 * PT:7 * PT])
                        ttadd(agg[:, :], tp[5][:, :], h1[:, 7 * PT:8 * PT])
                        if t == 8:
                            # seeds' layer-1 (l1a0 piece 8 reduced at u6)
                            psA = pp.tile([C, S], f32, tag="ps")
                            nc.tensor.matmul(psA, w["w2a"], l1a0[:, :],
                                             start=True, stop=False)
                            nc.tensor.matmul(psA, w["w2b"], h1self[:, 0:S],
                                             start=False, stop=True)
                            nc.scalar.activation(h2sb[:, 0:S], psA, Relu,
                                                 bias=bt["b2"])

            # ---------------- tail ----------------
            l1mm(8)
            l2self(8)
            red8(l2agg[:, 448:512], h2sb[:, 8 * PT: 9 * PT])
            # seeds' layer 2: full agg + self
            ps2 = pp.tile([C, S], f32, tag="ps")
            nc.tensor.matmul(ps2, w["w3a"], l2agg[:, :],
                             start=True, stop=False)
            nc.tensor.matmul(ps2, w["w3b"], h2sb[:, 0:S],
                             start=False, stop=True)
            nc.scalar.activation(ostage[:, 0:S], ps2, Relu, bias=bt["b3"])
            nc.sync.dma_start(out[:, 8 * PT:NPAR1], ostage[:, 8 * PT:NPAR1])
            nc.sync.dma_start(out[:, 0:S], ostage[:, 0:S])

    nc.compile()
    return nc


def _get_bass(key):
    if key not in _BASS_CACHE:
        _BASS_CACHE[key] = _build_bass_v2()
    return _BASS_CACHE[key]


def _edge_is_tree(edge):
    if edge.shape != (2, E_FULL):
        return False
    ar = np.arange(E_FULL, dtype=np.int64)
    return (np.array_equal(edge[0], (B + ar).astype(np.int32))
            and np.array_equal(edge[1], (ar // 8).astype(np.int32)))


def _fallback(x, edge, W1, b1, W2, b2, W3, b3):
    # General (structure-agnostic) CPU implementation; only used if the
    # inputs are not the fanout-8 tree this kernel is specialized for.
    sizes = [(N_FULL, E_FULL), (299008, 294912), (36864, 32768)]
    params = [(W1, b1), (W2, b2), (W3, b3)]
    x = x.astype(np.float32)
    for (n, e), (Wl, bl) in zip(sizes, params):
        src = edge[0, :e].astype(np.int64)
        dst = edge[1, :e].astype(np.int64)
        x = x[:n]
        agg = np.zeros((n, x.shape[1]), np.float32)
        np.add.at(agg, dst, x[src])
        deg = np.bincount(dst, minlength=n).astype(np.float32)
        agg /= np.maximum(deg, 1.0)[:, None]
        x = np.maximum(np.concatenate([agg, x], axis=1) @ Wl.T + bl, 0.0)
    return x


# pi permutation: hop2 local row 4096g + 8p + e  ->  4096g + 512e + p
# (de-interleave per 512-hop1-parent group, so the hop1-parent layer-0
# aggregation is 4 accumulating DR matmuls over contiguous plane pairs).
# Both reorders below are expressed as pure reshapes/transposes of that
# permutation (verified equivalent to the explicit index-gather form).


def kernel(**inputs):
    global LAST_RESULT
    import ml_dtypes

    x = np.asarray(inputs["x"])
    edge = np.asarray(inputs["edge"])
    W = [np.asarray(inputs[k], dtype=np.float32) for k in ("W1", "W2", "W3")]
    bias = [np.asarray(inputs[k], dtype=np.float32) for k in ("b1", "b2", "b3")]

    if x.shape != (N_FULL, C) or not _edge_is_tree(edge):
        return _fallback(x, edge, W[0], bias[0], W[1], bias[1], W[2], bias[2])

    from concourse.bass_utils import run_bass_kernel_spmd

    bf = ml_dtypes.bfloat16
    f8 = ml_dtypes.float8_e4m3fn          # bit-compatible with TRN e4m3 < 240
    x = np.ascontiguousarray(x, dtype=np.float32)

    wblocks = []
    for li in range(3):
        wblocks.append((W[li][:, :C] / 8.0).T)     # agg part, mean folded in
        wblocks.append(W[li][:, C:].T)             # self part
    # DoubleRow path: the fp8 agg weight (built on-device from w1a_raw,
    # scaled by DR_SCALE to sit in e4m3's normal range) pairs with a self
    # weight scaled by 8*DR_SCALE; the PSUM is divided back by 8*DR_SCALE
    # in the activation (ReLU is positively homogeneous), which also
    # restores the /8 of the mean.
    wblocks.append(W[0][:, C:].T * (8.0 * DR_SCALE))          # w1bs
    wblocks.append(W[0][:, :C].T)                             # w1a_raw
    wblocks.append(np.stack(bias, axis=1))                    # 3 bias cols
    wconsts = np.ascontiguousarray(np.concatenate(wblocks, axis=1)).astype(bf)

    in_maps = []
    for c in range(N_CORES):
        xloc = [x[OFF[h] + BLK[h] * c: OFF[h] + BLK[h] * (c + 1)]
                for h in range(4)]
        xAc = np.ascontiguousarray(np.concatenate(xloc[:2], axis=0).T).astype(bf)
        # hop2 in pi order
        xh2pi = xloc[2].reshape(8, PT, 8, C).transpose(0, 2, 1, 3)
        xh2c = np.ascontiguousarray(xh2pi.reshape(-1, C).T).astype(f8)
        # hop3 grouped by pi-ordered hop2 parent, then de-interleaved per
        # 512-parent group: col 4096k + 512e + p  <-  child e of pi-parent
        # 512k + p
        x3n = xloc[3].reshape(8, PT, 8, 8, C).transpose(0, 2, 3, 1, 4)
        x3c = np.ascontiguousarray(x3n.reshape(-1, C).T).astype(f8)
        in_maps.append({"xA": xAc, "xh2": xh2c, "x3": x3c,
                        "wconsts": wconsts})

    nc = _get_bass("v2")
    res = run_bass_kernel_spmd(nc, in_maps, list(range(N_CORES)), trace=TRACE)
    LAST_RESULT = res

    out = np.empty((OUT_ROWS, C), np.float32)
    for c in range(N_CORES):
        oc = np.asarray(res.results[c]["out"]).astype(np.float32)
        out[S * c: S * (c + 1)] = oc[:, :S].T
        out[B + 8 * S * c: B + 8 * S * (c + 1)] = oc[:, S:].T
    return out
